# revision 1
# baseline (speedup 1.0000x reference)
"""Grouped MLP (MoE expert-parallel) Trainium2 kernel.

Problem: hidden_states [131072, 1024] f32, 8 experts each owning a contiguous
16384-token block; per expert: SwiGLU MLP with gate_up [1024, 1024] and
down [512, 1024].

Sharding: expert-parallel — core e computes expert e's token block entirely
locally (no collectives). Inputs are sliced host-side, outputs concatenated.

Per-core kernel (per 128-token tile):
  - load x tile [128, 1024] f32 (HWDGE)
  - PE-transpose 8x [128,128] -> xT (f32r, rounded during PSUM->SBUF copy)
  - mm1: PSUM[128t, 512f] x2 accumulating over 8 h-slices;
         lhsT = xT[:,k,:] (stationary), rhs = W1[k, f-chunk] (f32r, resident)
  - SwiGLU: silu(gate) on ACT, * up on DVE (f32)
  - PE-transpose 4x -> hT (f32r)
  - mm2: PSUM[128t, 512]x2 over 4 i-slices with W2 resident
  - copy PSUM -> SBUF f32 out tile, DMA store (natural [tokens, H] layout)

float32r gives full PE rate (1 cyc/row at N=512) at ~1.5e-4 relative error.
"""

import numpy as np

E = 8
H = 1024
I = 512
T_PER_CORE = 16384
N_CORES = 8

_cache = {}


def _build_nc(n_tiles):
    import concourse.mybir as mybir
    import concourse.tile as tile
    from concourse import bacc
    from concourse.masks import make_identity

    f32 = mybir.dt.float32
    f32r = mybir.dt.float32r

    nc = bacc.Bacc(None, target_bir_lowering=False)
    n_tok = n_tiles * 128
    x = nc.dram_tensor("x", [n_tok, H], f32, kind="ExternalInput")
    w1 = nc.dram_tensor("w1", [H, 2 * I], f32, kind="ExternalInput")
    w2 = nc.dram_tensor("w2", [I, H], f32, kind="ExternalInput")
    out = nc.dram_tensor("out", [n_tok, H], f32, kind="ExternalOutput")

    with tile.TileContext(nc) as tc:
        with (
            tc.tile_pool(name="const", bufs=1) as const,
            tc.tile_pool(name="xin", bufs=4) as xin,
            tc.tile_pool(name="xtp", bufs=3) as xtp,
            tc.tile_pool(name="actp", bufs=4) as actp,
            tc.tile_pool(name="htp", bufs=3) as htp,
            tc.tile_pool(name="outp", bufs=3) as outp,
            tc.tile_pool(name="tp_ps", bufs=2, space="PSUM") as tp_ps_pool,
            tc.tile_pool(name="mm1_ps", bufs=4, space="PSUM") as mm1_ps_pool,
            tc.tile_pool(name="mm2_ps", bufs=2, space="PSUM") as mm2_ps_pool,
        ):
            # Resident weights, rounded to f32r during the load DMA (SWDGE cast).
            w1_sb = const.tile([128, H // 128, 2 * I], f32r)
            nc.gpsimd.dma_start(w1_sb[:], w1.ap().rearrange("(ho p) f -> p ho f", p=128))
            w2_sb = const.tile([128, I // 128, H], f32r)
            nc.gpsimd.dma_start(w2_sb[:], w2.ap().rearrange("(io p) f -> p io f", p=128))
            ident = const.tile([128, 128], f32)
            make_identity(nc, ident)

            # Software-pipelined emission. Per iteration i the PE stream is
            #   xtrans_i, mm1_{i-1}, mm2_{i-2}, htrans_{i-1}
            # so the SwiGLU chain of tile i-1 hides under mm2_{i-2}.
            xT_d, mm1_d, h_d, hT_d = {}, {}, {}, {}

            def stage_load_transpose(t):
                x_t = xin.tile([128, H], f32, tag="x")
                nc.sync.dma_start(x_t[:], x.ap()[t * 128 : (t + 1) * 128, :])
                xT = xtp.tile([128, H // 128, 128], f32r, tag="xT")
                for g in range(2):
                    tp_ps = tp_ps_pool.tile([128, 4, 128], f32, tag="tp")
                    for j in range(4):
                        k = g * 4 + j
                        nc.tensor.transpose(
                            tp_ps[:, j, :], x_t[:, k * 128 : (k + 1) * 128], ident
                        )
                    nc.scalar.copy(xT[:, g * 4 : (g + 1) * 4, :], tp_ps[:])
                xT_d[t] = xT

            def stage_mm1(t):
                xT = xT_d.pop(t)
                ps_pair = []
                for f in range(2):
                    ps = mm1_ps_pool.tile([128, 512], f32, tag="mm1")
                    for k in range(H // 128):
                        nc.tensor.matmul(
                            ps[:],
                            xT[:, k, :],
                            w1_sb[:, k, f * 512 : (f + 1) * 512],
                            start=(k == 0),
                            stop=(k == H // 128 - 1),
                        )
                    ps_pair.append(ps)
                mm1_d[t] = ps_pair

            def stage_swiglu(t):
                gate_ps, up_ps = mm1_d.pop(t)
                s = actp.tile([128, 512], f32, tag="s")
                nc.scalar.activation(
                    s[:], gate_ps[:], mybir.ActivationFunctionType.Sigmoid
                )
                t1 = actp.tile([128, 512], f32, tag="t1")
                nc.vector.tensor_mul(t1[:], s[:], up_ps[:])
                h = actp.tile([128, 512], f32, tag="h")
                nc.vector.tensor_mul(h[:], t1[:], gate_ps[:])
                h_d[t] = h

            def stage_htrans(t):
                h = h_d.pop(t)
                hT = htp.tile([128, I // 128, 128], f32r, tag="hT")
                tp_ps = tp_ps_pool.tile([128, 4, 128], f32, tag="tp")
                for k in range(4):
                    nc.tensor.transpose(
                        tp_ps[:, k, :], h[:, k * 128 : (k + 1) * 128], ident
                    )
                nc.scalar.copy(hT[:], tp_ps[:])
                hT_d[t] = hT

            def stage_mm2_store(t):
                hT = hT_d.pop(t)
                o_t = outp.tile([128, H], f32, tag="o")
                for f in range(2):
                    ps2 = mm2_ps_pool.tile([128, 512], f32, tag="mm2")
                    for k in range(I // 128):
                        nc.tensor.matmul(
                            ps2[:],
                            hT[:, k, :],
                            w2_sb[:, k, f * 512 : (f + 1) * 512],
                            start=(k == 0),
                            stop=(k == I // 128 - 1),
                        )
                    nc.vector.tensor_copy(o_t[:, f * 512 : (f + 1) * 512], ps2[:])
                nc.sync.dma_start(out.ap()[t * 128 : (t + 1) * 128, :], o_t[:])

            for i in range(n_tiles + 2):
                if i < n_tiles:
                    stage_load_transpose(i)
                if 1 <= i <= n_tiles:
                    stage_mm1(i - 1)
                    stage_swiglu(i - 1)
                if 2 <= i <= n_tiles + 1:
                    stage_mm2_store(i - 2)
                if 1 <= i <= n_tiles:
                    stage_htrans(i - 1)

    nc.compile()
    return nc


def _get_nc(n_tiles):
    if n_tiles not in _cache:
        _cache[n_tiles] = _build_nc(n_tiles)
    return _cache[n_tiles]


def kernel(hidden_states, gate_up_proj, down_proj, num_tokens_per_expert):
    sizes = np.asarray(num_tokens_per_expert)
    offsets = np.concatenate([[0], np.cumsum(sizes)])
    uniform = (
        sizes.shape[0] == E
        and np.all(sizes == T_PER_CORE)
        and hidden_states.shape == (E * T_PER_CORE, H)
    )
    if not uniform:
        # Fallback: host-side numpy (routing metadata other than the
        # compiled uniform case).
        outs = []
        for e in range(sizes.shape[0]):
            xe = hidden_states[offsets[e] : offsets[e + 1]].astype(np.float32)
            merged = xe @ gate_up_proj[e]
            gate, up = merged[:, :I], merged[:, I:]
            he = (gate / (1.0 + np.exp(-gate))) * up
            outs.append(he @ down_proj[e])
        return np.concatenate(outs, axis=0).astype(hidden_states.dtype)

    from concourse.bass_utils import run_bass_kernel_spmd

    nc = _get_nc(T_PER_CORE // 128)
    hs = np.ascontiguousarray(np.asarray(hidden_states, dtype=np.float32))
    w1 = np.ascontiguousarray(np.asarray(gate_up_proj, dtype=np.float32))
    w2 = np.ascontiguousarray(np.asarray(down_proj, dtype=np.float32))
    in_maps = [
        {
            "x": hs[e * T_PER_CORE : (e + 1) * T_PER_CORE],
            "w1": w1[e],
            "w2": w2[e],
        }
        for e in range(N_CORES)
    ]
    res = run_bass_kernel_spmd(nc, in_maps, core_ids=list(range(N_CORES)))
    return np.concatenate([r["out"] for r in res.results], axis=0)



# revision 2
# speedup vs baseline: 18.2034x; 18.2034x over previous
"""Grouped MLP (MoE expert-parallel) Trainium2 kernel.

Problem: hidden_states [131072, 1024] f32, 8 experts each owning a contiguous
16384-token block; per expert: SwiGLU MLP with gate_up [1024, 1024] and
down [512, 1024].

Sharding: expert-parallel - core e computes expert e's token block entirely
locally (no collectives). Inputs are sliced host-side, outputs concatenated.

v2 design (vs v1 which PE-transposed x and h and ran f32r):
  - bf16 compute (host-cast), f32 PSUM accumulation. rel-err ~0.3% << 2e-2.
  - x is loaded ALREADY TRANSPOSED via the HW xbar DMA-transpose (bf16-only
    feature) -> zero PE transposes for x.
  - mm1 runs feature-major: out1[f 128p, t 512] = W1[:,f-chunk].T @ xT,
    with W1 slices as the stationary operand. SwiGLU is elementwise in any
    layout, so h lands as h[i 128p, tokens] - which is exactly the lhsT
    (stationary) layout mm2 needs. mm2 emits token-major [tokens, H] so the
    store DMA is natural. -> zero PE transposes for h.
  - PE work per 512-token macro-tile: 64 mm1 + 32 mm2 matmuls of N=512,
    nothing else. Macro-tile-level software pipeline keeps PE dense.
"""

import numpy as np

E = 8
H = 1024
I = 512
T_PER_CORE = 16384
N_CORES = 8
TOK = 512  # macro-tile tokens

_cache = {}


def _build_nc(n_tok, reps=1, for_sim=False):
    import concourse.mybir as mybir
    import concourse.tile as tile
    from concourse import bacc

    f32 = mybir.dt.float32
    bf16 = mybir.dt.bfloat16
    # CoreSim doesn't implement Silu; HW does. silu(x) = x*sigmoid(x).
    Sigmoid = mybir.ActivationFunctionType.Sigmoid
    Silu = mybir.ActivationFunctionType.Silu

    nc = bacc.Bacc(None, target_bir_lowering=False)
    x = nc.dram_tensor("x", [n_tok, H], bf16, kind="ExternalInput")
    w1 = nc.dram_tensor("w1", [H, 2 * I], bf16, kind="ExternalInput")
    w2 = nc.dram_tensor("w2", [I, H], bf16, kind="ExternalInput")
    out = nc.dram_tensor("out", [n_tok, H], bf16, kind="ExternalOutput")

    n_mt = n_tok // TOK
    HC = H // 128  # 8 k-slices for mm1
    IC = I // 128  # 4 k-slices for mm2 / SwiGLU chunks
    TC = TOK // 128  # 4 token chunks per macro-tile

    with tile.TileContext(nc) as tc:
        with (
            tc.tile_pool(name="const", bufs=1) as const,
            tc.tile_pool(name="xtp", bufs=3) as xtp,
            tc.tile_pool(name="sp", bufs=3) as sp,
            tc.tile_pool(name="hp", bufs=2) as hp,
            tc.tile_pool(name="outp", bufs=3) as outp,
            tc.tile_pool(name="mm1_ps", bufs=4, space="PSUM") as mm1_ps,
            tc.tile_pool(name="mm2_ps", bufs=4, space="PSUM") as mm2_ps,
        ):
            # Resident weights in natural [k-slice, feature] layout.
            w1_sb = const.tile([128, HC, 2 * I], bf16)
            nc.sync.dma_start(w1_sb[:], w1.ap().rearrange("(hc p) f -> p hc f", p=128))
            w2_sb = const.tile([128, IC, H], bf16)
            nc.sync.dma_start(w2_sb[:], w2.ap().rearrange("(ic p) f -> p ic f", p=128))

            h_d = {}

            def stage_mm1_swiglu(mt):
                t0 = mt * TOK
                xT = xtp.tile([128, HC, TOK], bf16, tag="xT")
                for hc in range(HC):
                    nc.sync.dma_start(
                        xT[:, hc, :],
                        x.ap()[t0 : t0 + TOK, hc * 128 : (hc + 1) * 128],
                        transpose=True,
                    )
                h_sb = hp.tile([128, IC, TOK], bf16, tag="h")
                for ic in range(IC):
                    gate_ps = mm1_ps.tile([128, TOK], f32, tag="mm1")
                    for hc in range(HC):
                        nc.tensor.matmul(
                            gate_ps[:],
                            w1_sb[:, hc, ic * 128 : (ic + 1) * 128],
                            xT[:, hc, :],
                            start=(hc == 0),
                            stop=(hc == HC - 1),
                        )
                    up_ps = mm1_ps.tile([128, TOK], f32, tag="mm1")
                    for hc in range(HC):
                        nc.tensor.matmul(
                            up_ps[:],
                            w1_sb[:, hc, I + ic * 128 : I + (ic + 1) * 128],
                            xT[:, hc, :],
                            start=(hc == 0),
                            stop=(hc == HC - 1),
                        )
                    s_sb = sp.tile([128, TOK], f32, tag="s")
                    if for_sim:
                        nc.scalar.activation(s_sb[:], gate_ps[:], Sigmoid)
                        t_sb = sp.tile([128, TOK], f32, tag="t")
                        nc.vector.tensor_mul(t_sb[:], s_sb[:], gate_ps[:])
                        nc.vector.tensor_mul(h_sb[:, ic, :], t_sb[:], up_ps[:])
                    else:
                        nc.scalar.activation(s_sb[:], gate_ps[:], Silu)
                        nc.vector.tensor_mul(h_sb[:, ic, :], s_sb[:], up_ps[:])
                h_d[mt] = h_sb

            def stage_mm2_store(mt):
                t0 = mt * TOK
                h_sb = h_d.pop(mt)
                o_sb = outp.tile([128, TC, H], bf16, tag="o")
                for tci in range(TC):
                    pair = [
                        mm2_ps.tile([128, 512], f32, tag="mm2", name="ps2")
                        for _ in range(2)
                    ]
                    for ic in range(IC):
                        for oc in range(2):
                            nc.tensor.matmul(
                                pair[oc][:],
                                h_sb[:, ic, tci * 128 : (tci + 1) * 128],
                                w2_sb[:, ic, oc * 512 : (oc + 1) * 512],
                                start=(ic == 0),
                                stop=(ic == IC - 1),
                            )
                    for oc in range(2):
                        nc.vector.tensor_copy(
                            o_sb[:, tci, oc * 512 : (oc + 1) * 512], pair[oc][:]
                        )
                nc.sync.dma_start(
                    out.ap()[t0 : t0 + TOK, :].rearrange("(tc p) f -> p tc f", p=128),
                    o_sb[:],
                )

            for _rep in range(reps):
                for i in range(n_mt):
                    stage_mm1_swiglu(i)
                    if i >= 1:
                        stage_mm2_store(i - 1)
                stage_mm2_store(n_mt - 1)

    nc.compile()
    return nc


def _get_nc(n_tok):
    if n_tok not in _cache:
        _cache[n_tok] = _build_nc(n_tok)
    return _cache[n_tok]


def _to_bf16(a):
    import ml_dtypes

    return np.asarray(a, dtype=ml_dtypes.bfloat16)


def kernel(hidden_states, gate_up_proj, down_proj, num_tokens_per_expert):
    sizes = np.asarray(num_tokens_per_expert)
    offsets = np.concatenate([[0], np.cumsum(sizes)])
    uniform = (
        sizes.shape[0] == E
        and np.all(sizes == T_PER_CORE)
        and hidden_states.shape == (E * T_PER_CORE, H)
    )
    if not uniform:
        # Fallback: host-side numpy (routing metadata other than the
        # compiled uniform case).
        outs = []
        for e in range(sizes.shape[0]):
            xe = hidden_states[offsets[e] : offsets[e + 1]].astype(np.float32)
            merged = xe @ gate_up_proj[e]
            gate, up = merged[:, :I], merged[:, I:]
            he = (gate / (1.0 + np.exp(-gate))) * up
            outs.append(he @ down_proj[e])
        return np.concatenate(outs, axis=0).astype(hidden_states.dtype)

    from concourse.bass_utils import run_bass_kernel_spmd

    nc = _get_nc(T_PER_CORE)
    hs = _to_bf16(hidden_states)
    w1 = _to_bf16(gate_up_proj)
    w2 = _to_bf16(down_proj)
    in_maps = [
        {
            "x": np.ascontiguousarray(hs[e * T_PER_CORE : (e + 1) * T_PER_CORE]),
            "w1": np.ascontiguousarray(w1[e]),
            "w2": np.ascontiguousarray(w2[e]),
        }
        for e in range(N_CORES)
    ]
    res = run_bass_kernel_spmd(nc, in_maps, core_ids=list(range(N_CORES)))
    return np.concatenate([r["out"] for r in res.results], axis=0).astype(np.float32)


# revision 3
# speedup vs baseline: 20.7656x; 1.1408x over previous
"""Grouped MLP (MoE expert-parallel) Trainium2 kernel.

Problem: hidden_states [131072, 1024] f32, 8 experts each owning a contiguous
16384-token block; per expert: SwiGLU MLP with gate_up [1024, 1024] and
down [512, 1024].

Sharding: expert-parallel - core e computes expert e's token block entirely
locally (no collectives). Inputs are sliced host-side, outputs concatenated.

v2 design (vs v1 which PE-transposed x and h and ran f32r):
  - bf16 compute (host-cast), f32 PSUM accumulation. rel-err ~0.3% << 2e-2.
  - x is loaded ALREADY TRANSPOSED via the HW xbar DMA-transpose (bf16-only
    feature) -> zero PE transposes for x.
  - mm1 runs feature-major: out1[f 128p, t 512] = W1[:,f-chunk].T @ xT,
    with W1 slices as the stationary operand. SwiGLU is elementwise in any
    layout, so h lands as h[i 128p, tokens] - which is exactly the lhsT
    (stationary) layout mm2 needs. mm2 emits token-major [tokens, H] so the
    store DMA is natural. -> zero PE transposes for h.
  - PE work per 512-token macro-tile: 64 mm1 + 32 mm2 matmuls of N=512,
    nothing else. Macro-tile-level software pipeline keeps PE dense.
"""

import numpy as np

E = 8
H = 1024
I = 512
T_PER_CORE = 16384
N_CORES = 8
TOK = 512  # macro-tile tokens

_cache = {}


def _build_nc(n_tok, reps=1, for_sim=False):
    import concourse.mybir as mybir
    import concourse.tile as tile
    from concourse import bacc

    f32 = mybir.dt.float32
    bf16 = mybir.dt.bfloat16
    # CoreSim doesn't implement Silu; HW does. silu(x) = x*sigmoid(x).
    Sigmoid = mybir.ActivationFunctionType.Sigmoid
    Silu = mybir.ActivationFunctionType.Silu

    nc = bacc.Bacc(None, target_bir_lowering=False)
    x = nc.dram_tensor("x", [n_tok, H], bf16, kind="ExternalInput")
    w1 = nc.dram_tensor("w1", [H, 2 * I], bf16, kind="ExternalInput")
    w2 = nc.dram_tensor("w2", [I, H], bf16, kind="ExternalInput")
    out = nc.dram_tensor("out", [n_tok, H], bf16, kind="ExternalOutput")

    n_mt = n_tok // TOK
    HC = H // 128  # 8 k-slices for mm1
    IC = I // 128  # 4 k-slices for mm2 / SwiGLU chunks
    TC = TOK // 128  # 4 token chunks per macro-tile

    with tile.TileContext(nc) as tc:
        with (
            tc.tile_pool(name="const", bufs=1) as const,
            tc.tile_pool(name="xtp", bufs=3) as xtp,
            tc.tile_pool(name="sp", bufs=3) as sp,
            tc.tile_pool(name="hp", bufs=2) as hp,
            tc.tile_pool(name="outp", bufs=3) as outp,
            tc.tile_pool(name="mm1_ps", bufs=4, space="PSUM") as mm1_ps,
            tc.tile_pool(name="mm2_ps", bufs=4, space="PSUM") as mm2_ps,
        ):
            # Resident weights in natural [k-slice, feature] layout.
            w1_sb = const.tile([128, HC, 2 * I], bf16)
            nc.sync.dma_start(w1_sb[:], w1.ap().rearrange("(hc p) f -> p hc f", p=128))
            w2_sb = const.tile([128, IC, H], bf16)
            nc.sync.dma_start(w2_sb[:], w2.ap().rearrange("(ic p) f -> p ic f", p=128))

            h_d = {}

            def stage_mm1_swiglu(mt):
                t0 = mt * TOK
                xT = xtp.tile([128, HC, TOK], bf16, tag="xT")
                for hc in range(HC):
                    nc.sync.dma_start(
                        xT[:, hc, :],
                        x.ap()[t0 : t0 + TOK, hc * 128 : (hc + 1) * 128],
                        transpose=True,
                    )
                h_sb = hp.tile([128, IC, TOK], bf16, tag="h")
                for ic in range(IC):
                    gate_ps = mm1_ps.tile([128, TOK], f32, tag="mm1")
                    for hc in range(HC):
                        nc.tensor.matmul(
                            gate_ps[:],
                            w1_sb[:, hc, ic * 128 : (ic + 1) * 128],
                            xT[:, hc, :],
                            start=(hc == 0),
                            stop=(hc == HC - 1),
                        )
                    up_ps = mm1_ps.tile([128, TOK], f32, tag="mm1")
                    for hc in range(HC):
                        nc.tensor.matmul(
                            up_ps[:],
                            w1_sb[:, hc, I + ic * 128 : I + (ic + 1) * 128],
                            xT[:, hc, :],
                            start=(hc == 0),
                            stop=(hc == HC - 1),
                        )
                    s_sb = sp.tile([128, TOK], f32, tag="s")
                    if for_sim:
                        nc.scalar.activation(s_sb[:], gate_ps[:], Sigmoid)
                        t_sb = sp.tile([128, TOK], f32, tag="t")
                        nc.vector.tensor_mul(t_sb[:], s_sb[:], gate_ps[:])
                        nc.vector.tensor_mul(h_sb[:, ic, :], t_sb[:], up_ps[:])
                    else:
                        nc.scalar.activation(s_sb[:], gate_ps[:], Silu)
                        nc.vector.tensor_mul(h_sb[:, ic, :], s_sb[:], up_ps[:])
                h_d[mt] = h_sb

            def stage_mm2_store(mt):
                t0 = mt * TOK
                h_sb = h_d.pop(mt)
                o_sb = outp.tile([128, TC, H], bf16, tag="o")
                for tci in range(TC):
                    pair = [
                        mm2_ps.tile([128, 512], f32, tag="mm2", name="ps2")
                        for _ in range(2)
                    ]
                    for ic in range(IC):
                        for oc in range(2):
                            nc.tensor.matmul(
                                pair[oc][:],
                                h_sb[:, ic, tci * 128 : (tci + 1) * 128],
                                w2_sb[:, ic, oc * 512 : (oc + 1) * 512],
                                start=(ic == 0),
                                stop=(ic == IC - 1),
                            )
                    for oc in range(2):
                        nc.vector.tensor_copy(
                            o_sb[:, tci, oc * 512 : (oc + 1) * 512], pair[oc][:]
                        )
                nc.sync.dma_start(
                    out.ap()[t0 : t0 + TOK, :].rearrange("(tc p) f -> p tc f", p=128),
                    o_sb[:],
                )

            for _rep in range(reps):
                for i in range(n_mt):
                    stage_mm1_swiglu(i)
                    if i >= 1:
                        stage_mm2_store(i - 1)
                stage_mm2_store(n_mt - 1)

    nc.compile()
    return nc


def _get_nc(n_tok):
    if n_tok not in _cache:
        _cache[n_tok] = _build_nc(n_tok)
    return _cache[n_tok]


def _to_bf16(a):
    import ml_dtypes

    return np.asarray(a, dtype=ml_dtypes.bfloat16)


def kernel(hidden_states, gate_up_proj, down_proj, num_tokens_per_expert):
    sizes = np.asarray(num_tokens_per_expert)
    offsets = np.concatenate([[0], np.cumsum(sizes)])
    uniform = (
        sizes.shape[0] == E
        and np.all(sizes == T_PER_CORE)
        and hidden_states.shape == (E * T_PER_CORE, H)
    )
    if not uniform:
        # Fallback: host-side numpy (routing metadata other than the
        # compiled uniform case).
        hs_np = np.asarray(hidden_states, dtype=np.float32)
        w1_np = np.asarray(gate_up_proj, dtype=np.float32)
        w2_np = np.asarray(down_proj, dtype=np.float32)
        outs = []
        for e in range(sizes.shape[0]):
            xe = hs_np[offsets[e] : offsets[e + 1]]
            merged = xe @ w1_np[e]
            gate, up = merged[:, :I], merged[:, I:]
            he = (gate / (1.0 + np.exp(-gate))) * up
            outs.append(he @ w2_np[e])
        return np.concatenate(outs, axis=0).astype(np.asarray(hidden_states).dtype)

    from concourse.bass_utils import run_bass_kernel_spmd

    nc = _get_nc(T_PER_CORE)
    hs = _to_bf16(hidden_states)
    w1 = _to_bf16(gate_up_proj)
    w2 = _to_bf16(down_proj)
    in_maps = [
        {
            "x": np.ascontiguousarray(hs[e * T_PER_CORE : (e + 1) * T_PER_CORE]),
            "w1": np.ascontiguousarray(w1[e]),
            "w2": np.ascontiguousarray(w2[e]),
        }
        for e in range(N_CORES)
    ]
    res = run_bass_kernel_spmd(nc, in_maps, core_ids=list(range(N_CORES)))
    return np.concatenate([r["out"] for r in res.results], axis=0).astype(np.float32)


# revision 5
# speedup vs baseline: 39.0564x; 1.8808x over previous
"""Grouped MLP (MoE expert-parallel) Trainium2 kernel.

Problem: hidden_states [131072, 1024] f32, 8 experts each owning a contiguous
16384-token block; per expert: SwiGLU MLP with gate_up [1024, 1024] and
down [512, 1024].

Sharding: expert-parallel - core e computes expert e's token block entirely
locally (no collectives). Inputs are sliced host-side, outputs concatenated.

v2 design (vs v1 which PE-transposed x and h and ran f32r):
  - bf16 compute (host-cast), f32 PSUM accumulation. rel-err ~0.3% << 2e-2.
  - x is loaded ALREADY TRANSPOSED via the HW xbar DMA-transpose (bf16-only
    feature) -> zero PE transposes for x.
  - mm1 runs feature-major: out1[f 128p, t 512] = W1[:,f-chunk].T @ xT,
    with W1 slices as the stationary operand. SwiGLU is elementwise in any
    layout, so h lands as h[i 128p, tokens] - which is exactly the lhsT
    (stationary) layout mm2 needs. mm2 emits token-major [tokens, H] so the
    store DMA is natural. -> zero PE transposes for h.
  - PE work per 512-token macro-tile: 64 mm1 + 32 mm2 matmuls of N=512,
    nothing else. Macro-tile-level software pipeline keeps PE dense.
"""

import numpy as np

E = 8
H = 1024
I = 512
T_PER_CORE = 16384
N_CORES = 8
TOK = 512  # macro-tile tokens

_cache = {}


def _build_nc(n_tok, reps=1, for_sim=False):
    import concourse.mybir as mybir
    import concourse.tile as tile
    from concourse import bacc

    f32 = mybir.dt.float32
    bf16 = mybir.dt.bfloat16
    # CoreSim doesn't implement Silu; HW does. silu(x) = x*sigmoid(x).
    Sigmoid = mybir.ActivationFunctionType.Sigmoid
    Silu = mybir.ActivationFunctionType.Silu

    nc = bacc.Bacc(None, target_bir_lowering=False)
    x = nc.dram_tensor("x", [n_tok, H], bf16, kind="ExternalInput")
    w1 = nc.dram_tensor("w1", [H, 2 * I], bf16, kind="ExternalInput")
    w2 = nc.dram_tensor("w2", [I, H], bf16, kind="ExternalInput")
    out = nc.dram_tensor("out", [n_tok, H], bf16, kind="ExternalOutput")

    n_mt = n_tok // TOK
    HC = H // 128  # 8 k-slices for mm1
    IC = I // 128  # 4 k-slices for mm2 / SwiGLU chunks
    TC = TOK // 128  # 4 token chunks per macro-tile

    with tile.TileContext(nc) as tc:
        with (
            tc.tile_pool(name="const", bufs=1) as const,
            tc.tile_pool(name="xtp", bufs=4) as xtp,
            tc.tile_pool(name="sp", bufs=4) as sp,
            tc.tile_pool(name="hp", bufs=3) as hp,
            tc.tile_pool(name="outp", bufs=3) as outp,
            tc.tile_pool(name="mm1_ps", bufs=4, space="PSUM") as mm1_ps,
            tc.tile_pool(name="mm2_ps", bufs=4, space="PSUM") as mm2_ps,
        ):
            # Resident weights in natural [k-slice, feature] layout.
            w1_sb = const.tile([128, HC, 2 * I], bf16)
            nc.sync.dma_start(w1_sb[:], w1.ap().rearrange("(hc p) f -> p hc f", p=128))
            w2_sb = const.tile([128, IC, H], bf16)
            nc.sync.dma_start(w2_sb[:], w2.ap().rearrange("(ic p) f -> p ic f", p=128))

            h_d = {}

            def stage_mm1_swiglu(mt):
                t0 = mt * TOK
                xT = xtp.tile([128, HC, TOK], bf16, tag="xT")
                for hc in range(HC):
                    nc.sync.dma_start(
                        xT[:, hc, :],
                        x.ap()[t0 : t0 + TOK, hc * 128 : (hc + 1) * 128],
                        transpose=True,
                    )
                h_sb = hp.tile([128, IC, TOK], bf16, tag="h")
                for ic in range(IC):
                    gate_ps = mm1_ps.tile([128, TOK], f32, tag="mm1")
                    for hc in range(HC):
                        nc.tensor.matmul(
                            gate_ps[:],
                            w1_sb[:, hc, ic * 128 : (ic + 1) * 128],
                            xT[:, hc, :],
                            start=(hc == 0),
                            stop=(hc == HC - 1),
                        )
                    up_ps = mm1_ps.tile([128, TOK], f32, tag="mm1")
                    for hc in range(HC):
                        nc.tensor.matmul(
                            up_ps[:],
                            w1_sb[:, hc, I + ic * 128 : I + (ic + 1) * 128],
                            xT[:, hc, :],
                            start=(hc == 0),
                            stop=(hc == HC - 1),
                        )
                    s_sb = sp.tile([128, TOK], f32, tag="s")
                    if for_sim:
                        nc.scalar.activation(s_sb[:], gate_ps[:], Sigmoid)
                        t_sb = sp.tile([128, TOK], f32, tag="t")
                        nc.vector.tensor_mul(t_sb[:], s_sb[:], gate_ps[:])
                        nc.vector.tensor_mul(h_sb[:, ic, :], t_sb[:], up_ps[:])
                    else:
                        nc.scalar.activation(s_sb[:], gate_ps[:], Silu)
                        nc.vector.tensor_mul(h_sb[:, ic, :], s_sb[:], up_ps[:])
                h_d[mt] = h_sb

            def stage_mm2_store(mt):
                t0 = mt * TOK
                h_sb = h_d.pop(mt)
                o_sb = outp.tile([128, TC, H], bf16, tag="o")
                for tci in range(TC):
                    pair = [
                        mm2_ps.tile([128, 512], f32, tag="mm2", name="ps2")
                        for _ in range(2)
                    ]
                    for ic in range(IC):
                        for oc in range(2):
                            nc.tensor.matmul(
                                pair[oc][:],
                                h_sb[:, ic, tci * 128 : (tci + 1) * 128],
                                w2_sb[:, ic, oc * 512 : (oc + 1) * 512],
                                start=(ic == 0),
                                stop=(ic == IC - 1),
                            )
                    for oc in range(2):
                        nc.vector.tensor_copy(
                            o_sb[:, tci, oc * 512 : (oc + 1) * 512], pair[oc][:]
                        )
                nc.sync.dma_start(
                    out.ap()[t0 : t0 + TOK, :].rearrange("(tc p) f -> p tc f", p=128),
                    o_sb[:],
                )

            # PE order per iteration: mm2(i-1) first (inputs already on-chip),
            # then mm1(i) (needs the xT DMA) - gives the DMA-transpose loads a
            # full macro-tile of slack before PE consumes them.
            prev = None
            for _rep in range(reps):
                for i in range(n_mt):
                    if prev is not None:
                        stage_mm2_store(*prev)
                    stage_mm1_swiglu(i)
                    prev = (i,)
            stage_mm2_store(*prev)

    nc.compile()
    return nc


def _get_nc(n_tok):
    if n_tok not in _cache:
        _cache[n_tok] = _build_nc(n_tok)
    return _cache[n_tok]


def _to_bf16(a):
    import ml_dtypes

    return np.asarray(a, dtype=ml_dtypes.bfloat16)


def kernel(hidden_states, gate_up_proj, down_proj, num_tokens_per_expert):
    sizes = np.asarray(num_tokens_per_expert)
    offsets = np.concatenate([[0], np.cumsum(sizes)])
    uniform = (
        sizes.shape[0] == E
        and np.all(sizes == T_PER_CORE)
        and hidden_states.shape == (E * T_PER_CORE, H)
    )
    if not uniform:
        # Fallback: host-side numpy (routing metadata other than the
        # compiled uniform case).
        hs_np = np.asarray(hidden_states, dtype=np.float32)
        w1_np = np.asarray(gate_up_proj, dtype=np.float32)
        w2_np = np.asarray(down_proj, dtype=np.float32)
        outs = []
        for e in range(sizes.shape[0]):
            xe = hs_np[offsets[e] : offsets[e + 1]]
            merged = xe @ w1_np[e]
            gate, up = merged[:, :I], merged[:, I:]
            he = (gate / (1.0 + np.exp(-gate))) * up
            outs.append(he @ w2_np[e])
        return np.concatenate(outs, axis=0).astype(np.asarray(hidden_states).dtype)

    from concourse.bass_utils import run_bass_kernel_spmd

    nc = _get_nc(T_PER_CORE)
    hs = _to_bf16(hidden_states)
    w1 = _to_bf16(gate_up_proj)
    w2 = _to_bf16(down_proj)
    in_maps = [
        {
            "x": np.ascontiguousarray(hs[e * T_PER_CORE : (e + 1) * T_PER_CORE]),
            "w1": np.ascontiguousarray(w1[e]),
            "w2": np.ascontiguousarray(w2[e]),
        }
        for e in range(N_CORES)
    ]
    res = run_bass_kernel_spmd(nc, in_maps, core_ids=list(range(N_CORES)))
    return np.concatenate([r["out"] for r in res.results], axis=0).astype(np.float32)
